# revision 22
# baseline (speedup 1.0000x reference)
"""Trainium2 Bass kernel for nn_Normalizer (annealed top-k masking normalizer).

Math (see reference): the 20-iteration annealed loop converges to the fixed
point of  c = s(c)/k,  s(c) = sum_i min(E_i, c),  E_i = exp(sm_i/theta),
theta = 0.3 (the last 12 reference iterations run at constant theta and
forget the annealing path).  gamma = min(E/c*, 1).

v5 design (latency-shaped pipeline, slope-free r=2 relaxation):
  - host: sm = where(mask==0, -60000, score) in fp16 (one 8MB/core input).
  - ACT queue: E0,k0,E1,k1,copy0,E2,k2,copy1,E3,k3,copy2 -- only Exp/Copy
    (one table load).  E = exp(sm/theta) bf16 + accum s_inf.  Exact
    k-count = accum of exp(sm/10000) (unmasked -> exactly 1.0 in bf16,
    masked -> exactly e^-6).  Copy+accum row-sums min-pass outputs for
    tiles 0-2; tile 3's second eval is a fused DVE min+accum so the last
    row-sum does not serialize on ACT at the tail.
  - DVE per-tile chain depends only on E(j): subsample count -> k_hat
    (exact k arrives later from ACT and is used from the first full eval
    onward), c0 = s_inf/k_hat, 2 subsample evals, fused full eval s1,
    relaxed update, 4x-mode bf16 min pass for s2.  All [P,1] scalar math
    stays on DVE (AP-scalar tensor_scalar + reciprocal) -- no cross-engine
    ping-pong.
  - updates over-relaxed with r=2 (contraction |1-2(1-lam)|, lam~0.55):
      c' = (s*rk)^2/c  -- pure multiplies.
  Validated: l2 rel err ~1.5e-3 vs f32 reference (gate 2e-2).

Sharding: pure row-parallel, 4096 rows -> 8 cores x 512 rows (4 tiles of
[128, 8192] per core).
"""

import math
import os
import sys

import numpy as np

try:
    import concourse.bass as bass  # noqa: F401
except ImportError:
    sys.path.insert(0, "/opt/trn_rl_repo")
    import concourse.bass as bass  # noqa: F401

import ml_dtypes

import concourse.bacc as bacc
import concourse.tile as tile
from concourse import mybir
from concourse.bass_utils import run_bass_kernel_spmd

F32 = mybir.dt.float32
BF16 = mybir.dt.bfloat16
FP16 = mybir.dt.float16
A = mybir.AluOpType
AF = mybir.ActivationFunctionType

# Problem constants
THETA, P_FRAC = 0.3, 0.1
BSZ, SEQ = 4096, 8192
N_CORES = 8
ROWS_PER_CORE = BSZ // N_CORES          # 512
P = 128                                  # partitions
N_TILES = ROWS_PER_CORE // P             # 4
CHUNK = 16                               # subsample: 16 cols every 128
CHUNK_EVERY = 128
N_CHUNKS = SEQ // CHUNK_EVERY            # 64
SUB = N_CHUNKS * CHUNK                   # 1024
SUB_SCALE = float(SEQ // SUB)            # 8
PEN = -60000.0                           # fp16-representable mask penalty
KTEMP = 10000.0                          # k-count exp temperature
EM6 = math.exp(PEN / KTEMP)              # e^-6
KC1 = -float(SEQ) * EM6
KC2 = P_FRAC / (1.0 - EM6)               # k = (S + KC1) * KC2

N_SUB_ITERS = int(os.environ.get("NORM_SUB_ITERS", "1"))
KHAT = 0.1 * 0.5 * SEQ                   # 409.6: expected per-row k


def _sub_view(ap):
    """[P, SEQ] access pattern -> [P, N_CHUNKS, CHUNK] strided view."""
    return ap.rearrange("p (c l) -> p c l", l=CHUNK_EVERY)[:, :, 0:CHUNK]


def build_kernel():
    nc = bacc.Bacc("TRN2", target_bir_lowering=False, debug=False,
                   num_devices=N_CORES)
    sm_d = nc.dram_tensor("sm", [ROWS_PER_CORE, SEQ], FP16,
                          kind="ExternalInput")
    gamma_d = nc.dram_tensor("gamma", [ROWS_PER_CORE, SEQ], BF16,
                             kind="ExternalOutput")

    NT = N_TILES
    with tile.TileContext(nc) as tc:
        with (
            tc.tile_pool(name="smp", bufs=1) as smp,
            tc.tile_pool(name="ep", bufs=1) as ep,
            tc.tile_pool(name="j2p", bufs=1) as j2p,
            tc.tile_pool(name="gjp", bufs=1) as gjp,
            tc.tile_pool(name="jkp", bufs=1) as jkp,
            tc.tile_pool(name="j1p", bufs=1) as j1p,
            tc.tile_pool(name="sjp", bufs=1) as sjp,
            tc.tile_pool(name="scal", bufs=1) as scal,
        ):
            # shared write-only junk outputs
            jk = jkp.tile([P, SEQ], BF16, name="jka", tag="jka")   # ACT junk
            j1 = j1p.tile([P, SEQ], BF16, name="j1d", tag="j1d")   # DVE junk
            sj = sjp.tile([P, SUB], BF16, name="sjd", tag="sjd")   # sub junk

            sm = [None] * NT
            e_t = [None] * NT
            sinf = [None] * NT
            sc_t = [None] * NT
            rk = [None] * NT
            rk2 = [None] * NT
            c_t = [None] * NT
            j2 = [None] * NT
            s2 = [None] * NT

            def ts(out, in0, s1v, s2v, op0, op1=A.bypass, accum=None):
                nc.vector.tensor_scalar(out=out, in0=in0, scalar1=s1v,
                                        scalar2=s2v, op0=op0, op1=op1,
                                        accum_out=accum)

            def new_scal(nm):
                return scal.tile([P, 1], F32, name=nm, tag=nm)

            def dve_relax(j, c_prev, s_val, rksq, tagp):
                """c_new = (s_val^2 * rksq) / c_prev, all on DVE (keeping
                the chain on one engine beats offloading: cross-engine sem
                latency exceeds the ~0.5us of DVE time)."""
                rq = rksq if isinstance(rksq, float) else rksq[:]
                u2 = new_scal(f"{tagp}u2_{j}")
                ts(u2[:], s_val[:], s_val[:], rq, A.mult, A.mult)
                rcp = new_scal(f"{tagp}rcp_{j}")
                nc.vector.reciprocal(out=rcp[:], in_=c_prev[:])
                c_new = new_scal(f"{tagp}c_{j}")
                ts(c_new[:], u2[:], rcp[:], None, A.mult)
                return c_new

            def emit_act(j):
                e_t[j] = ep.tile([P, SEQ], BF16, name=f"E{j}", tag=f"E{j}")
                sinf[j] = new_scal(f"sinf{j}")
                nc.scalar.activation(out=e_t[j][:], in_=sm[j][:],
                                     func=AF.Exp, scale=1.0 / THETA,
                                     accum_out=sinf[j][:])
                sc_t[j] = new_scal(f"sc{j}")
                nc.scalar.activation(out=jk[:], in_=sm[j][:], func=AF.Exp,
                                     scale=1.0 / KTEMP,
                                     accum_out=sc_t[j][:])

            def emit_chain(j):
                # constant k-hat seeds the pre-full-eval phase (rows' true
                # k is 410 +- ~4%; the full evals with exact k absorb it)
                c_t[j] = new_scal(f"c0_{j}")
                ts(c_t[j][:], sinf[j][:], 1.0 / KHAT, None, A.mult)
                # subsample relaxed evals
                for t in range(N_SUB_ITERS):
                    ss = new_scal(f"ss{t}_{j}")
                    ts(sj[:].rearrange("p (c l) -> p c l", l=CHUNK),
                       _sub_view(e_t[j][:]), c_t[j][:], None, A.min, A.add,
                       accum=ss[:])
                    c_t[j] = dve_relax(j, c_t[j], ss,
                                       (SUB_SCALE / KHAT) ** 2, f"s{t}")
                # full eval 1 (fused min+accum)
                s1 = new_scal(f"s1_{j}")
                ts(j1[:], e_t[j][:], c_t[j][:], None, A.min, A.add,
                   accum=s1[:])
                # exact k consts (ACT kexp(j) accum is long since ready)
                k_t = new_scal(f"k{j}")
                ts(k_t[:], sc_t[j][:], KC1, KC2, A.add, A.mult)
                rk[j] = new_scal(f"rk{j}")
                nc.vector.reciprocal(out=rk[j][:], in_=k_t[:])
                rk2[j] = new_scal(f"rk2{j}")
                ts(rk2[j][:], rk[j][:], rk[j][:], None, A.mult)
                c_t[j] = dve_relax(j, c_t[j], s1, rk2[j], "n1")
                # second eval: min pass (4x) summed by ACT copy (tiles 0-2)
                # or fused on DVE (tile 3)
                s2[j] = new_scal(f"s2_{j}")
                if j < 2:
                    j2[j] = j2p.tile([P, SEQ], BF16, name=f"j2_{j % 2}",
                                     tag=f"j2_{j % 2}")
                    ts(j2[j][:], e_t[j][:], c_t[j][:], None, A.min)
                else:
                    ts(j1[:], e_t[j][:], c_t[j][:], None, A.min, A.add,
                       accum=s2[j][:])

            def emit_copy(j):
                nc.scalar.activation(out=jk[:], in_=j2[j][:], func=AF.Copy,
                                     accum_out=s2[j][:])

            def emit_tail(j):
                c3 = dve_relax(j, c_t[j], s2[j], rk2[j], "n2")
                rc = new_scal(f"rc{j}")
                nc.vector.reciprocal(out=rc[:], in_=c3[:])
                gj = gjp.tile([P, SEQ], BF16, name=f"gj{j % 2}",
                              tag=f"gj{j % 2}")
                ts(gj[:], e_t[j][:], rc[:], 1.0, A.mult, A.min)
                nc.sync.dma_start(out=gamma_d.ap()[j * P:(j + 1) * P, :],
                                  in_=gj[:])

            # input DMAs
            for j in range(NT):
                sm[j] = smp.tile([P, SEQ], FP16, name=f"sm{j % 2}",
                                 tag=f"sm{j % 2}")
                nc.sync.dma_start(out=sm[j][:],
                                  in_=sm_d.ap()[j * P:(j + 1) * P, :])

            # interleaved emission: ACT gets E0,k0,E1,k1,cp0,E2,k2,cp1,
            # E3,k3,cp2; DVE gets chain0..chain3 then tails.
            emit_act(0)
            emit_chain(0)
            emit_act(1)
            emit_chain(1)
            emit_copy(0)
            emit_act(2)
            emit_chain(2)
            emit_copy(1)
            emit_act(3)
            emit_chain(3)
            for j in range(NT):
                emit_tail(j)

    nc.compile()
    return nc


_NC_CACHE = None


def encode_sm(score: np.ndarray, mask: np.ndarray) -> np.ndarray:
    """Pre-masked score in fp16: masked entries -> -60000."""
    sm = np.where(np.asarray(mask) == 0, np.float32(PEN),
                  np.asarray(score, dtype=np.float32))
    return sm.astype(np.float16)


def kernel(score: np.ndarray, mask: np.ndarray) -> np.ndarray:
    global _NC_CACHE
    if _NC_CACHE is None:
        _NC_CACHE = build_kernel()
    nc = _NC_CACHE

    sm = encode_sm(score, mask)
    in_maps = []
    for i in range(N_CORES):
        sl = slice(i * ROWS_PER_CORE, (i + 1) * ROWS_PER_CORE)
        in_maps.append({"sm": np.ascontiguousarray(sm[sl])})
    res = run_bass_kernel_spmd(nc, in_maps, core_ids=list(range(N_CORES)))
    out = np.concatenate([res.results[i]["gamma"] for i in range(N_CORES)],
                         axis=0)
    return out.astype(np.float32)


# revision 24
# speedup vs baseline: 1.0338x; 1.0338x over previous
"""Trainium2 Bass kernel for nn_Normalizer (annealed top-k masking normalizer).

Math (see reference): the 20-iteration annealed loop converges to the fixed
point of  c = s(c)/k,  s(c) = sum_i min(E_i, c),  E_i = exp(sm_i/theta),
theta = 0.3 (the last 12 reference iterations run at constant theta and
forget the annealing path).  gamma = min(E/c*, 1).

v5 design (latency-shaped pipeline, slope-free r=2 relaxation):
  - host: sm = where(mask==0, -60000, score) in fp16 (one 8MB/core input).
  - ACT queue: E0,k0,E1,k1,copy0,E2,k2,copy1,E3,k3,copy2 -- only Exp/Copy
    (one table load).  E = exp(sm/theta) bf16 + accum s_inf.  Exact
    k-count = accum of exp(sm/10000) (unmasked -> exactly 1.0 in bf16,
    masked -> exactly e^-6).  Copy+accum row-sums min-pass outputs for
    tiles 0-2; tile 3's second eval is a fused DVE min+accum so the last
    row-sum does not serialize on ACT at the tail.
  - DVE per-tile chain depends only on E(j): subsample count -> k_hat
    (exact k arrives later from ACT and is used from the first full eval
    onward), c0 = s_inf/k_hat, 2 subsample evals, fused full eval s1,
    relaxed update, 4x-mode bf16 min pass for s2.  All [P,1] scalar math
    stays on DVE (AP-scalar tensor_scalar + reciprocal) -- no cross-engine
    ping-pong.
  - updates over-relaxed with r=2 (contraction |1-2(1-lam)|, lam~0.55):
      c' = (s*rk)^2/c  -- pure multiplies.
  Validated: l2 rel err ~1.5e-3 vs f32 reference (gate 2e-2).

Sharding: pure row-parallel, 4096 rows -> 8 cores x 512 rows (4 tiles of
[128, 8192] per core).
"""

import math
import os
import sys

import numpy as np

try:
    import concourse.bass as bass  # noqa: F401
except ImportError:
    sys.path.insert(0, "/opt/trn_rl_repo")
    import concourse.bass as bass  # noqa: F401

import ml_dtypes

import concourse.bacc as bacc
import concourse.tile as tile
from concourse import mybir
from concourse.bass_utils import run_bass_kernel_spmd

F32 = mybir.dt.float32
BF16 = mybir.dt.bfloat16
FP16 = mybir.dt.float16
A = mybir.AluOpType
AF = mybir.ActivationFunctionType

# Problem constants
THETA, P_FRAC = 0.3, 0.1
BSZ, SEQ = 4096, 8192
N_CORES = 8
ROWS_PER_CORE = BSZ // N_CORES          # 512
P = 128                                  # partitions
N_TILES = ROWS_PER_CORE // P             # 4
CHUNK = 16                               # subsample: 16 cols every 128
CHUNK_EVERY = 128
N_CHUNKS = SEQ // CHUNK_EVERY            # 64
SUB = N_CHUNKS * CHUNK                   # 1024
SUB_SCALE = float(SEQ // SUB)            # 8
PEN = -60000.0                           # fp16-representable mask penalty
KTEMP = 10000.0                          # k-count exp temperature
EM6 = math.exp(PEN / KTEMP)              # e^-6
KC1 = -float(SEQ) * EM6
KC2 = P_FRAC / (1.0 - EM6)               # k = (S + KC1) * KC2

N_SUB_ITERS = int(os.environ.get("NORM_SUB_ITERS", "1"))
KHAT = 0.1 * 0.5 * SEQ                   # 409.6: expected per-row k


def _sub_view(ap):
    """[P, SEQ] access pattern -> [P, N_CHUNKS, CHUNK] strided view."""
    return ap.rearrange("p (c l) -> p c l", l=CHUNK_EVERY)[:, :, 0:CHUNK]


def build_kernel():
    nc = bacc.Bacc("TRN2", target_bir_lowering=False, debug=False,
                   num_devices=N_CORES)
    sm_d = nc.dram_tensor("sm", [ROWS_PER_CORE, SEQ], FP16,
                          kind="ExternalInput")
    gamma_d = nc.dram_tensor("gamma", [ROWS_PER_CORE, SEQ], BF16,
                             kind="ExternalOutput")

    NT = N_TILES
    with tile.TileContext(nc) as tc:
        with (
            tc.tile_pool(name="smp", bufs=1) as smp,
            tc.tile_pool(name="ep", bufs=1) as ep,
            tc.tile_pool(name="j2p", bufs=1) as j2p,
            tc.tile_pool(name="gjp", bufs=1) as gjp,
            tc.tile_pool(name="jkp", bufs=1) as jkp,
            tc.tile_pool(name="j1p", bufs=1) as j1p,
            tc.tile_pool(name="sjp", bufs=1) as sjp,
            tc.tile_pool(name="scal", bufs=1) as scal,
        ):
            # shared write-only junk outputs
            jk = jkp.tile([P, SEQ], BF16, name="jka", tag="jka")   # ACT junk
            j1 = j1p.tile([P, SEQ], BF16, name="j1d", tag="j1d")   # DVE junk
            sj = sjp.tile([P, SUB], BF16, name="sjd", tag="sjd")   # sub junk

            sm = [None] * NT
            e_t = [None] * NT
            sinf = [None] * NT
            sc_t = [None] * NT
            rk = [None] * NT
            rk2 = [None] * NT
            c_t = [None] * NT
            j2 = [None] * NT
            s2 = [None] * NT

            def ts(out, in0, s1v, s2v, op0, op1=A.bypass, accum=None):
                nc.vector.tensor_scalar(out=out, in0=in0, scalar1=s1v,
                                        scalar2=s2v, op0=op0, op1=op1,
                                        accum_out=accum)

            def new_scal(nm):
                return scal.tile([P, 1], F32, name=nm, tag=nm)

            def dve_relax(j, c_prev, s_val, rksq, tagp):
                """c_new = (s_val^2 * rksq) / c_prev, all on DVE (keeping
                the chain on one engine beats offloading: cross-engine sem
                latency exceeds the ~0.5us of DVE time)."""
                rq = rksq if isinstance(rksq, float) else rksq[:]
                u2 = new_scal(f"{tagp}u2_{j}")
                ts(u2[:], s_val[:], s_val[:], rq, A.mult, A.mult)
                rcp = new_scal(f"{tagp}rcp_{j}")
                nc.vector.reciprocal(out=rcp[:], in_=c_prev[:])
                c_new = new_scal(f"{tagp}c_{j}")
                ts(c_new[:], u2[:], rcp[:], None, A.mult)
                return c_new

            def emit_act(j):
                e_t[j] = ep.tile([P, SEQ], BF16, name=f"E{j}", tag=f"E{j}")
                sinf[j] = new_scal(f"sinf{j}")
                if j == 0:
                    H = SEQ // 2
                    sA = new_scal("sinf0a")
                    nc.scalar.activation(out=e_t[0][:, 0:H],
                                         in_=sm[0][:, 0:H], func=AF.Exp,
                                         scale=1.0 / THETA, accum_out=sA[:])
                    sB = new_scal("sinf0b")
                    nc.scalar.activation(out=e_t[0][:, H:SEQ],
                                         in_=sm[0][:, H:SEQ], func=AF.Exp,
                                         scale=1.0 / THETA, accum_out=sB[:])
                    ts(sinf[0][:], sA[:], sB[:], None, A.add)
                else:
                    nc.scalar.activation(out=e_t[j][:], in_=sm[j][:],
                                         func=AF.Exp, scale=1.0 / THETA,
                                         accum_out=sinf[j][:])
                sc_t[j] = new_scal(f"sc{j}")
                nc.scalar.activation(out=jk[:], in_=sm[j][:], func=AF.Exp,
                                     scale=1.0 / KTEMP,
                                     accum_out=sc_t[j][:])

            def emit_chain(j):
                # constant k-hat seeds the pre-full-eval phase (rows' true
                # k is 410 +- ~4%; the full evals with exact k absorb it)
                c_t[j] = new_scal(f"c0_{j}")
                ts(c_t[j][:], sinf[j][:], 1.0 / KHAT, None, A.mult)
                # subsample relaxed evals
                for t in range(N_SUB_ITERS):
                    ss = new_scal(f"ss{t}_{j}")
                    ts(sj[:].rearrange("p (c l) -> p c l", l=CHUNK),
                       _sub_view(e_t[j][:]), c_t[j][:], None, A.min, A.add,
                       accum=ss[:])
                    c_t[j] = dve_relax(j, c_t[j], ss,
                                       (SUB_SCALE / KHAT) ** 2, f"s{t}")
                # full eval 1 (fused min+accum)
                s1 = new_scal(f"s1_{j}")
                ts(j1[:], e_t[j][:], c_t[j][:], None, A.min, A.add,
                   accum=s1[:])
                # exact k consts (ACT kexp(j) accum is long since ready)
                k_t = new_scal(f"k{j}")
                ts(k_t[:], sc_t[j][:], KC1, KC2, A.add, A.mult)
                rk[j] = new_scal(f"rk{j}")
                nc.vector.reciprocal(out=rk[j][:], in_=k_t[:])
                rk2[j] = new_scal(f"rk2{j}")
                ts(rk2[j][:], rk[j][:], rk[j][:], None, A.mult)
                c_t[j] = dve_relax(j, c_t[j], s1, rk2[j], "n1")
                # second eval: min pass (4x) summed by ACT copy (tiles 0-2)
                # or fused on DVE (tile 3)
                s2[j] = new_scal(f"s2_{j}")
                if j % 2 == 0:
                    j2[j] = j2p.tile([P, SEQ], BF16, name=f"j2_{j % 2}",
                                     tag=f"j2_{j % 2}")
                    ts(j2[j][:], e_t[j][:], c_t[j][:], None, A.min)
                else:
                    ts(j1[:], e_t[j][:], c_t[j][:], None, A.min, A.add,
                       accum=s2[j][:])

            def emit_copy(j):
                nc.scalar.activation(out=jk[:], in_=j2[j][:], func=AF.Copy,
                                     accum_out=s2[j][:])

            def emit_tail(j):
                c3 = dve_relax(j, c_t[j], s2[j], rk2[j], "n2")
                rc = new_scal(f"rc{j}")
                nc.vector.reciprocal(out=rc[:], in_=c3[:])
                gj = gjp.tile([P, SEQ], BF16, name=f"gj{j % 2}",
                              tag=f"gj{j % 2}")
                ts(gj[:], e_t[j][:], rc[:], 1.0, A.mult, A.min)
                nc.sync.dma_start(out=gamma_d.ap()[j * P:(j + 1) * P, :],
                                  in_=gj[:])

            # input DMAs (tile 0 split in halves so E0 starts early)
            H = SEQ // 2
            sm[0] = smp.tile([P, SEQ], FP16, name="sm0", tag="sm0")
            nc.sync.dma_start(out=sm[0][:, 0:H], in_=sm_d.ap()[0:P, 0:H])
            nc.sync.dma_start(out=sm[0][:, H:SEQ],
                              in_=sm_d.ap()[0:P, H:SEQ])
            for j in range(1, NT):
                sm[j] = smp.tile([P, SEQ], FP16, name=f"sm{j % 2}",
                                 tag=f"sm{j % 2}")
                nc.sync.dma_start(out=sm[j][:],
                                  in_=sm_d.ap()[j * P:(j + 1) * P, :])

            # interleaved emission: ACT gets E0,k0,E1,k1,cp0,E2,k2,cp1,
            # E3,k3,cp2; DVE gets chain0..chain3 then tails.
            emit_act(0)
            emit_chain(0)
            emit_act(1)
            emit_chain(1)
            emit_copy(0)
            emit_act(2)
            emit_chain(2)
            emit_act(3)
            emit_chain(3)
            emit_copy(2)
            for j in (0, 1, 3, 2):
                emit_tail(j)

    nc.compile()
    return nc


_NC_CACHE = None


def encode_sm(score: np.ndarray, mask: np.ndarray) -> np.ndarray:
    """Pre-masked score in fp16: masked entries -> -60000."""
    sm = np.where(np.asarray(mask) == 0, np.float32(PEN),
                  np.asarray(score, dtype=np.float32))
    return sm.astype(np.float16)


def kernel(score: np.ndarray, mask: np.ndarray) -> np.ndarray:
    global _NC_CACHE
    if _NC_CACHE is None:
        _NC_CACHE = build_kernel()
    nc = _NC_CACHE

    sm = encode_sm(score, mask)
    in_maps = []
    for i in range(N_CORES):
        sl = slice(i * ROWS_PER_CORE, (i + 1) * ROWS_PER_CORE)
        in_maps.append({"sm": np.ascontiguousarray(sm[sl])})
    res = run_bass_kernel_spmd(nc, in_maps, core_ids=list(range(N_CORES)))
    out = np.concatenate([res.results[i]["gamma"] for i in range(N_CORES)],
                         axis=0)
    return out.astype(np.float32)
